# revision 24
# baseline (speedup 1.0000x reference)
"""Trainium2 Bass kernel for a 4-layer transformer encoder.

Model (hardcoded from the problem spec):
  L=4 layers, B=4, S=2048, D=512, H=8 heads (DH=64), FF=2048,
  inference BatchNorm with moving stats (0,1): bn(x) = x/sqrt(1+eps)*g + be.

Sharding: 8 cores. Cores (2b, 2b+1) handle batch item b; each computes
1024 of the item's 2048 tokens.  The residual stream is kept
feature-major [D, T] in SBUF (f32); matmul operands are bf16.  After
each layer the pair exchanges its updated bf16 activations via a
2-rank AllGather through DRAM bounce buffers so both cores have the
full 2048-token sequence for the next layer's K/V.  Attention scores
are computed transposed [ktok, qtok] (K=DH=64 contraction, two heads
row-packed into the 128-row PE array), exp on ScalarE (scale folded
in; logits are O(1) so no max subtraction), and AV uses a ones-
augmented V (M=65) so the softmax denominator comes out of the same
matmul; the reciprocal row is partition-broadcast with a small DMA and
applied on DVE.
"""

import math

import numpy as np
import ml_dtypes

# ---- problem constants --------------------------------------------------
L, B, S, D, H = 4, 4, 2048, 512, 8
DH = D // H            # 64
FF = 4 * D             # 2048
BN_EPS = 1e-3
P = 128
DT = D // P            # 4 feature tiles
FT = FF // P           # 16 ffn tiles
ATT_SCALE = 1.0 / math.sqrt(DH)
BN_INV = 1.0 / math.sqrt(1.0 + BN_EPS)

N_CORES = 8

BF16 = ml_dtypes.bfloat16


def _fmajor(a, t):
    """[T, D_any] -> feature-major [128, D_any//128, T] tile layout."""
    d = a.shape[1]
    return np.ascontiguousarray(a.T.reshape(d // P, P, t).transpose(1, 0, 2))


def _w_tiles(w):
    """[K, N] weight -> [128, K//128, N] (partition = K within tile)."""
    k, n = w.shape
    return np.ascontiguousarray(w.reshape(k // P, P, n).transpose(1, 0, 2))


def _vec_tiles(v):
    """[L, D_any] -> [L, 128, D_any//128] per-partition layout."""
    l, d = v.shape
    return np.ascontiguousarray(v.reshape(l, d // P, P).transpose(0, 2, 1))


def build_encoder(nc, tc, *, n_layers, t_own, s_kv, use_ag):
    """Emit the encoder onto TileContext tc.  t_own = tokens this core
    computes; s_kv = tokens attended over (= 2*t_own when use_ag)."""
    import concourse.bass as bass
    import concourse.mybir as mybir
    from concourse.bass import ds, ts

    F32 = mybir.dt.float32
    BF = mybir.dt.bfloat16
    AF = mybir.ActivationFunctionType
    OP = mybir.AluOpType

    KT = s_kv // P          # ktok 128-tiles
    SKT = s_kv // 512       # kv projection stream tiles
    QT = t_own // 512       # qtok stream tiles
    HP = H // 2             # head pairs

    # ---- dram I/O -------------------------------------------------------
    x0_d = nc.dram_tensor("x0", [P, DT, t_own], F32, kind="ExternalInput").ap()
    xkv0_d = nc.dram_tensor("xkv0", [P, DT, s_kv], BF, kind="ExternalInput").ap()
    wq_d = nc.dram_tensor("wq", [n_layers, P, DT, D], BF, kind="ExternalInput").ap()
    wk_d = nc.dram_tensor("wk", [n_layers, P, DT, D], BF, kind="ExternalInput").ap()
    wv_d = nc.dram_tensor("wv", [n_layers, P, DT, D], BF, kind="ExternalInput").ap()
    wo_d = nc.dram_tensor("wo", [n_layers, DH, H, D], BF, kind="ExternalInput").ap()
    w1_d = nc.dram_tensor("w1", [n_layers, P, DT, FF], BF, kind="ExternalInput").ap()
    w2_d = nc.dram_tensor("w2", [n_layers, P, FT, D], BF, kind="ExternalInput").ap()
    vecs_d = nc.dram_tensor("vecs", [6, n_layers, P, DT], F32, kind="ExternalInput").ap()
    b1_d = nc.dram_tensor("b1v", [n_layers, P, FT], F32, kind="ExternalInput").ap()
    bv_d = nc.dram_tensor("bvt", [n_layers, D], F32, kind="ExternalInput").ap()
    out_d = nc.dram_tensor("out", [P, DT, t_own], F32, kind="ExternalOutput").ap()

    import contextlib
    stack = contextlib.ExitStack()

    state = stack.enter_context(tc.tile_pool(name="state", bufs=1))
    wsmall = stack.enter_context(tc.tile_pool(name="wsmall", bufs=1))
    w1pool = stack.enter_context(tc.tile_pool(name="w1pool", bufs=1))
    w2pool = stack.enter_context(tc.tile_pool(name="w2pool", bufs=1))
    epool = stack.enter_context(tc.tile_pool(name="epool", bufs=4))
    hpool = stack.enter_context(tc.tile_pool(name="hpool", bufs=FT + 2))
    dpool = stack.enter_context(tc.tile_pool(name="dpool", bufs=2))
    bvpool = stack.enter_context(tc.tile_pool(name="bvpool", bufs=2))
    rpool = stack.enter_context(tc.tile_pool(name="rpool", bufs=3))
    scpool = stack.enter_context(tc.tile_pool(name="scpool", bufs=2, space="PSUM"))
    avpool = stack.enter_context(tc.tile_pool(name="avpool", bufs=3, space="PSUM"))
    mmpool = stack.enter_context(tc.tile_pool(name="mmpool", bufs=2, space="PSUM"))
    ftpool = stack.enter_context(tc.tile_pool(name="ftpool", bufs=1, space="PSUM"))
    dram = stack.enter_context(tc.tile_pool(name="dram", bufs=2, space="DRAM"))

    # ---- persistent state ----------------------------------------------
    x_sb = state.tile([P, DT, t_own], F32)       # residual stream (f32)
    xbf_a = state.tile([P, DT, t_own], BF)       # post-BN2 cast (qkv rhs / exchange)
    xbf_b = state.tile([P, DT, t_own], BF)       # post-BN1 cast (ffn rhs)
    kvx = state.tile([P, DT, s_kv], BF)          # kv-source activations (full seq)
    kT = state.tile([P, DT, s_kv], BF)           # K, feature-major
    q_sb = state.tile([P, DT, t_own], BF)        # Q, feature-major
    vplus = state.tile([P, KT, H, DH + 1], BF)   # V token-major + ones col
    attnT = state.tile([DH, H, t_own], BF)       # attention out, [dh, head, tok]

    vecs_sb = state.tile([P, 6, n_layers, DT], F32)
    nc.sync.dma_start(vecs_sb[:], vecs_d.rearrange("v l p f -> p v l f"))
    b1_sb = state.tile([P, n_layers, FT], F32)
    nc.sync.dma_start(b1_sb[:], b1_d.rearrange("l p f -> p l f"))

    BQ, BK, S1, BE1, S2, BE2 = range(6)

    nc.vector.memset(vplus[:, :, :, DH : DH + 1], 1.0)

    def layer(l):
        # ---- weights for this layer ---------------------------------
        wq_sb = wsmall.tile([P, DT, D], BF, tag="wq")
        nc.sync.dma_start(wq_sb[:], wq_d[l])
        wk_sb = wsmall.tile([P, DT, D], BF, tag="wk")
        nc.sync.dma_start(wk_sb[:], wk_d[l])
        wv_sb = wsmall.tile([P, DT, D], BF, tag="wv")
        nc.sync.dma_start(wv_sb[:], wv_d[l])
        wo_sb = wsmall.tile([DH, H, D], BF, tag="wo")
        nc.sync.dma_start(wo_sb[:], wo_d[l])
        w1_sb = w1pool.tile([P, DT, FF], BF, tag="w1")
        nc.sync.dma_start(w1_sb[:], w1_d[l])
        w2_sb = w2pool.tile([P, FT, D], BF, tag="w2")
        nc.sync.dma_start(w2_sb[:], w2_d[l])
        bvrow = bvpool.tile([1, D], F32, tag="bvrow")
        nc.sync.dma_start(bvrow[:], bv_d[l][None, :])
        bvb = bvpool.tile([P, D], F32, tag="bvb")
        nc.gpsimd.partition_broadcast(bvb[:], bvrow[:])

        # ---- kv-source for this layer -------------------------------
        if l == 0:
            nc.sync.dma_start(kvx[:], xkv0_d)
        elif use_ag:
            # one AG per qt-half so the first half's exchange overlaps the
            # previous layer's second-half FFN
            for qt in range(QT):
                qsl = slice(qt * 512, (qt + 1) * 512)
                bounce_in = dram.tile([P, DT, 512], BF, tag="agin", name=f"agin{qt}")
                bounce_out = dram.tile([2, P, DT, 512], BF, tag="agout", name=f"agout{qt}")
                nc.sync.dma_start(bounce_in[:], xbf_a[:, :, qsl])
                nc.gpsimd.collective_compute(
                    "AllGather",
                    mybir.AluOpType.bypass,
                    replica_groups=[[0, 1], [2, 3], [4, 5], [6, 7]],
                    ins=[bounce_in[:].opt()],
                    outs=[bounce_out[:].opt()],
                )
                for s in range(2):
                    nc.sync.dma_start(
                        kvx[:, :, s * t_own + qt * 512 : s * t_own + (qt + 1) * 512],
                        bounce_out[s],
                    )
        else:
            nc.vector.tensor_copy(kvx[:], xbf_a[:])

        # ---- projection piece emitters ------------------------------
        def kproj_piece(dt, st):
            ps = mmpool.tile([P, 512], F32, tag="mm", name=f"kp{dt}_{st}")
            for kd in range(DT):
                nc.tensor.matmul(
                    ps[:],
                    wk_sb[:, kd, dt * P : (dt + 1) * P],
                    kvx[:, kd, st * 512 : (st + 1) * 512],
                    start=(kd == 0),
                    stop=(kd == DT - 1),
                )
            nc.vector.tensor_scalar(
                kT[:, dt, st * 512 : (st + 1) * 512],
                ps[:],
                vecs_sb[:, BK, l, dt : dt + 1],
                None,
                OP.add,
            )

        def vproj_piece(tt):
            ps = mmpool.tile([P, 512], F32, tag="mm", name=f"vp{tt}")
            for kd in range(DT):
                nc.tensor.matmul(
                    ps[:],
                    kvx[:, kd, tt * P : (tt + 1) * P],
                    wv_sb[:, kd, :],
                    start=(kd == 0),
                    stop=(kd == DT - 1),
                )
            nc.vector.tensor_tensor(
                vplus[:, tt, :, 0:DH],
                ps[:].rearrange("p (h e) -> p h e", h=H),
                bvb[:].rearrange("p (h e) -> p h e", h=H),
                OP.add,
            )

        def qproj_piece(dt, st):
            ps = mmpool.tile([P, 512], F32, tag="mm", name=f"qp{dt}_{st}")
            for kd in range(DT):
                nc.tensor.matmul(
                    ps[:],
                    wq_sb[:, kd, dt * P : (dt + 1) * P],
                    xbf_a[:, kd, st * 512 : (st + 1) * 512],
                    start=(kd == 0),
                    stop=(kd == DT - 1),
                )
            nc.vector.tensor_scalar(
                q_sb[:, dt, st * 512 : (st + 1) * 512],
                ps[:],
                vecs_sb[:, BQ, l, dt : dt + 1],
                None,
                OP.add,
            )

        def oproj_piece(qt, dt):
            qsl = slice(qt * 512, (qt + 1) * 512)
            ps = mmpool.tile([P, 512], F32, tag="mm", name=f"op{qt}_{dt}")
            for h in range(H):
                nc.tensor.matmul(
                    ps[:],
                    wo_sb[:, h, dt * P : (dt + 1) * P],
                    attnT[:, h, qsl],
                    start=(h == 0),
                    stop=(h == H - 1),
                )
            nc.vector.tensor_tensor(
                x_sb[:, dt, qsl], x_sb[:, dt, qsl], ps[:], OP.add
            )
            nc.vector.tensor_scalar(
                x_sb[:, dt, qsl],
                x_sb[:, dt, qsl],
                vecs_sb[:, S1, l, dt : dt + 1],
                vecs_sb[:, BE1, l, dt : dt + 1],
                OP.mult,
                OP.add,
            )
            nc.vector.tensor_copy(xbf_b[:, dt, qsl], x_sb[:, dt, qsl])

        hsbs = {}

        def ht_piece(qt, ft):
            qsl = slice(qt * 512, (qt + 1) * 512)
            hps = mmpool.tile([P, 512], F32, tag="mm", name=f"hp{qt}_{ft}")
            for kd in range(DT):
                nc.tensor.matmul(
                    hps[:],
                    w1_sb[:, kd, ft * P : (ft + 1) * P],
                    xbf_b[:, kd, qsl],
                    start=(kd == 0),
                    stop=(kd == DT - 1),
                )
            hsb = hpool.tile([P, 512], BF, tag="h", name=f"h{qt}_{ft}")
            nc.vector.tensor_scalar(
                hsb[:], hps[:], b1_sb[:, l, ft : ft + 1], 0.0, OP.add, OP.max
            )
            hsbs[(qt, ft)] = hsb

        def ft_piece(qt, dt):
            qsl = slice(qt * 512, (qt + 1) * 512)
            fps = ftpool.tile([P, 512], F32, tag="ft", name=f"ft{qt}_{dt}")
            for ft in range(FT):
                nc.tensor.matmul(
                    fps[:],
                    w2_sb[:, ft, dt * P : (dt + 1) * P],
                    hsbs[(qt, ft)][:],
                    start=(ft == 0),
                    stop=(ft == FT - 1),
                )
            nc.vector.tensor_tensor(
                x_sb[:, dt, qsl], x_sb[:, dt, qsl], fps[:], OP.add
            )
            nc.vector.tensor_scalar(
                x_sb[:, dt, qsl],
                x_sb[:, dt, qsl],
                vecs_sb[:, S2, l, dt : dt + 1],
                vecs_sb[:, BE2, l, dt : dt + 1],
                OP.mult,
                OP.add,
            )
            nc.vector.tensor_copy(xbf_a[:, dt, qsl], x_sb[:, dt, qsl])

        # ---- attention chunk for one (qt, hp), woven with filler ----
        def attn_chunk(qt, hp, filler, per_kt, deadlines=False):
            qsl = slice(qt * 512, (qt + 1) * 512)
            av0 = avpool.tile([P, 512], F32, tag="av", name=f"av0_{qt}_{hp}")
            av1 = avpool.tile([P, 512], F32, tag="av", name=f"av1_{qt}_{hp}")
            budget = 0.0
            for kt in range(KT):
                if deadlines:
                    key = hp * KT + kt
                    while filler and filler[0][0] is not None and filler[0][0] <= key + 2:
                        filler.pop(0)[1]()
                ps0 = scpool.tile([P, 512], F32, tag="sc", name=f"sc0_{qt}_{hp}_{kt}")
                ps1 = scpool.tile([P, 512], F32, tag="sc", name=f"sc1_{qt}_{hp}_{kt}")
                nc.tensor.matmul(
                    ps0[:],
                    kT[0:DH, hp, kt * P : (kt + 1) * P],
                    q_sb[0:DH, hp, qsl],
                    start=True,
                    stop=True,
                )
                nc.tensor.matmul(
                    ps1[:],
                    kT[DH:P, hp, kt * P : (kt + 1) * P],
                    q_sb[DH:P, hp, qsl],
                    start=True,
                    stop=True,
                )
                e0 = epool.tile([P, 512], BF, tag="e", name=f"e0_{qt}_{hp}_{kt}")
                nc.scalar.activation(e0[:], ps0[:], AF.Exp, scale=ATT_SCALE)
                e1 = epool.tile([P, 512], BF, tag="e", name=f"e1_{qt}_{hp}_{kt}")
                nc.scalar.activation(e1[:], ps1[:], AF.Exp, scale=ATT_SCALE)
                nc.tensor.matmul(
                    av0[0 : DH + 1],
                    vplus[:, kt, 2 * hp, :],
                    e0[:],
                    start=(kt == 0),
                    stop=(kt == KT - 1),
                )
                nc.tensor.matmul(
                    av1[0 : DH + 1],
                    vplus[:, kt, 2 * hp + 1, :],
                    e1[:],
                    start=(kt == 0),
                    stop=(kt == KT - 1),
                )
                budget += per_kt
                while budget >= 1.0 and filler:
                    filler.pop(0)[1]()
                    budget -= 1.0
            den = dpool.tile([DH + 1, 2, 512], BF, tag="den", name=f"dn{qt}_{hp}")
            with nc.allow_low_precision(reason="softmax denom bf16"):
                nc.vector.reciprocal(den[DH : DH + 1, 0, :], av0[DH : DH + 1, :])
                nc.vector.reciprocal(den[DH : DH + 1, 1, :], av1[DH : DH + 1, :])
            den0 = dpool.tile([1, 2, 512], BF, tag="den0", name=f"d0{qt}_{hp}")
            nc.sync.dma_start(den0[:], den[DH : DH + 1, :, :])
            rb = rpool.tile([DH, 2, 512], BF, tag="rb", name=f"rb{qt}_{hp}")
            nc.gpsimd.partition_broadcast(rb[:], den0[:])
            for j, av in ((0, av0), (1, av1)):
                h = 2 * hp + j
                nc.vector.tensor_tensor(
                    attnT[:, h, qsl], av[0:DH, :], rb[:, j, :], OP.mult
                )

        from functools import partial

        # lead-in: K/Q for head-pair 0, V for the first token tiles
        kproj_piece(0, 0)
        kproj_piece(0, 1)
        qproj_piece(0, 0)
        vproj_piece(0)
        vproj_piece(1)

        # filler for attention(qt0): remaining K/Q/V projections, each
        # tagged with the (hp*KT + kt) step of attention(qt0) that first
        # needs it (None = not needed until attention(qt1)).
        fill0 = []
        for tt in range(2, KT):
            fill0.append((tt, partial(vproj_piece, tt)))
        for dt in range(DT):
            for st in range(SKT):
                if dt == 0 and st < 2:
                    continue
                fill0.append((dt * KT + st * (KT // SKT), partial(kproj_piece, dt, st)))
            if dt > 0:
                fill0.append((dt * KT, partial(qproj_piece, dt, 0)))
        if QT > 1:
            for dt in range(DT):
                fill0.append((None, partial(qproj_piece, dt, 1)))
        fill0.sort(key=lambda t: t[0] if t[0] is not None else 10 ** 9)

        per_kt0 = max(len(fill0) / (HP * KT), 0.01)
        for hp in range(HP):
            attn_chunk(0, hp, fill0, per_kt0, deadlines=True)
        for _, f in fill0:
            f()
        fill0.clear()

        # attention(qt1) woven with o-proj + FFN of qt0
        if QT > 1:
            fill1 = [(None, partial(oproj_piece, 0, dt)) for dt in range(DT)]
            for ft in range(FT):
                fill1.append((None, partial(ht_piece, 0, ft)))
            fill1 += [(None, partial(ft_piece, 0, dt)) for dt in range(DT)]
            per_kt1 = len(fill1) / (HP * KT)
            for hp in range(HP):
                attn_chunk(1, hp, fill1, per_kt1)
            for _, f in fill1:
                f()
            fill1.clear()

        # tail: o-proj + FFN of the last qt
        last = QT - 1
        for dt in range(DT):
            oproj_piece(last, dt)
        for ft in range(FT):
            ht_piece(last, ft)
        for dt in range(DT):
            ft_piece(last, dt)

    # initial load + cast
    nc.sync.dma_start(x_sb[:], x0_d)
    nc.vector.tensor_copy(xbf_a[:], x_sb[:])

    for l in range(n_layers):
        layer(l)

    nc.sync.dma_start(out_d, x_sb[:])
    stack.close()


def _host_inputs(sequence, wq, bq, wk, bk, wv, bv, wo, bo, w1, b1, w2, b2,
                 g1, be1, g2, be2, *, n_layers=L, t_own=S // 2, s_kv=S,
                 use_ag=True, n_cores=N_CORES):
    """Build the shared + per-core input maps."""
    s1 = (g1 * BN_INV).astype(np.float32)
    be1p = (bo * s1 + be1).astype(np.float32)
    s2 = (g2 * BN_INV).astype(np.float32)
    be2p = (b2 * s2 + be2).astype(np.float32)

    vecs = np.stack([
        _vec_tiles(bq), _vec_tiles(bk),
        _vec_tiles(s1), _vec_tiles(be1p),
        _vec_tiles(s2), _vec_tiles(be2p),
    ]).astype(np.float32)                        # [6, L, 128, DT]

    shared = {
        "wq": np.stack([_w_tiles(wq[l]) for l in range(n_layers)]).astype(BF16),
        "wk": np.stack([_w_tiles(wk[l]) for l in range(n_layers)]).astype(BF16),
        "wv": np.stack([_w_tiles(wv[l]) for l in range(n_layers)]).astype(BF16),
        "wo": np.stack([
            wo[l].reshape(H, DH, D).transpose(1, 0, 2) for l in range(n_layers)
        ]).astype(BF16),
        "w1": np.stack([_w_tiles(w1[l]) for l in range(n_layers)]).astype(BF16),
        "w2": np.stack([_w_tiles(w2[l]) for l in range(n_layers)]).astype(BF16),
        "vecs": vecs,
        "b1v": _vec_tiles(b1).astype(np.float32),
        "bvt": bv.astype(np.float32),
    }

    in_maps = []
    for i in range(n_cores):
        if use_ag:
            b, half = i // 2, i % 2
            tok = slice(half * t_own, (half + 1) * t_own)
        else:
            b, tok = i % sequence.shape[0], slice(0, t_own)
        m = dict(shared)
        m["x0"] = _fmajor(sequence[b][tok].astype(np.float32), t_own)
        m["xkv0"] = _fmajor(sequence[b][:s_kv], s_kv).astype(BF16)
        in_maps.append(m)
    return in_maps


def _assemble(results, *, t_own=S // 2, use_ag=True):
    out = np.zeros((B, S, D), np.float32)
    for i, r in enumerate(results):
        xo = r["out"]                        # [128, DT, t_own]
        xd = xo.transpose(1, 0, 2).reshape(DT * P, t_own).T   # [t_own, D]
        if use_ag:
            b, half = i // 2, i % 2
            out[b, half * t_own : (half + 1) * t_own] = xd
        else:
            if i < B:
                out[i, :t_own] = xd
    return out


def _build(n_layers=L, t_own=S // 2, s_kv=S, use_ag=True, n_cores=N_CORES):
    from concourse import bacc
    import concourse.tile as tile

    nc = bacc.Bacc(
        "TRN2",
        target_bir_lowering=False,
        debug=False,
        enable_asserts=False,
        num_devices=n_cores,
    )
    with tile.TileContext(nc) as tc:
        build_encoder(nc, tc, n_layers=n_layers, t_own=t_own, s_kv=s_kv,
                      use_ag=use_ag)
    nc.compile()
    return nc


def kernel(**inputs) -> np.ndarray:
    from concourse.bass_utils import run_bass_kernel_spmd

    use_ag = True
    t_own = S // 2
    nc = _build(use_ag=use_ag, t_own=t_own)
    in_maps = _host_inputs(**{k: np.asarray(v) for k, v in inputs.items()},
                           use_ag=use_ag, t_own=t_own)
    res = run_bass_kernel_spmd(nc, in_maps, core_ids=list(range(N_CORES)))
    return _assemble(res.results, t_own=t_own, use_ag=use_ag)
